# revision 8
# baseline (speedup 1.0000x reference)
"""Trainium2 Bass kernel: CACE-style GNN message passing (nn_Cace_7155415515517).

Strategy (node-parallel, no collectives needed):
  - Host: sort edges by receiver node, shard receivers across 8 cores
    (1280 nodes each), lay edges out in 128-edge chunks grouped into
    128-node blocks (12 chunks per block, padded with null edges).
  - Device per core: compute radial basis (Chebyshev sin recurrence),
    angular monomials, z-masked radial x angular payload P[e, 320];
    segment-sum via one-hot matmul into PSUM per node block; symmetrize
    using the factorization A[n,r,a,c1,c2] = emb[n,c2] * M[n,r,a,c1],
    so B_l = (sum_a pref*M^2) * emb^2 needs only per-node work.
  - sqrt(pref) multinomial prefactors are folded into the angular basis.
"""
import math
import numpy as np

import concourse.bacc as bacc
import concourse.bass as bass
import concourse.mybir as mybir
import concourse.tile as tile

AF = mybir.ActivationFunctionType
ALU = mybir.AluOpType
F32 = mybir.dt.float32
BF16 = mybir.dt.bfloat16

N_CORES = 8
N_NODES = 10000
N_RBF = 8
NPC = 1280            # nodes per core (8*1280 = 10240, tail padded)
NBLK = 10             # 128-node blocks per core
CPB = 12              # 128-edge chunks per block
NCH = NBLK * CPB      # 120 chunks -> 15360 edge slots per core
CUT = 5.5
SQ2C = math.sqrt(2.0 / CUT)
S2, S3, S6 = math.sqrt(2.0), math.sqrt(3.0), math.sqrt(6.0)

_CACHE = {}


def _build():
    nc = bacc.Bacc("TRN2", target_bir_lowering=False, debug=False,
                   num_devices=N_CORES)
    r_d = nc.dram_tensor("r", [128, NCH], F32, kind="ExternalInput")
    v_d = nc.dram_tensor("v3", [128, 3 * NCH], F32, kind="ExternalInput")
    z_d = nc.dram_tensor("z1", [128, NCH], F32, kind="ExternalInput")
    d_d = nc.dram_tensor("drel", [128, NCH], BF16, kind="ExternalInput")
    i_d = nc.dram_tensor("iota", [128, 128], BF16, kind="ExternalInput")
    e_d = nc.dram_tensor("emb", [128, 3 * NBLK], F32, kind="ExternalInput")
    w_d = nc.dram_tensor("wbc", [128, 6], F32, kind="ExternalInput")
    o_d = nc.dram_tensor("out", [128, 288 * NBLK], F32, kind="ExternalOutput")

    with tile.TileContext(nc) as tc:
        with (
            tc.tile_pool(name="mp", bufs=1) as mp,
            tc.tile_pool(name="pp", bufs=4) as pp,
            tc.tile_pool(name="ps", bufs=2, space="PSUM") as ps,
        ):
            # ---- input loads ----
            r = mp.tile([128, NCH], F32, tag="r")
            nc.sync.dma_start(r[:], r_d.ap())
            v = mp.tile([128, 3, NCH], F32, tag="v")
            nc.sync.dma_start(v[:], v_d.ap().rearrange("p (a c) -> p a c", a=3))
            z1 = mp.tile([128, NCH], F32, tag="z1")
            nc.sync.dma_start(z1[:], z_d.ap())
            drel = mp.tile([128, NCH], BF16, tag="drel")
            nc.sync.dma_start(drel[:], d_d.ap())
            iot = mp.tile([128, 128], BF16, tag="iota")
            nc.sync.dma_start(iot[:], i_d.ap())
            emb = mp.tile([128, NBLK, 3], F32, tag="emb")
            nc.sync.dma_start(emb[:], e_d.ap().rearrange("p (b c) -> p b c", b=NBLK))
            wbc = mp.tile([128, 6], F32, tag="wbc")
            nc.sync.dma_start(wbc[:], w_d.ap())

            one = mp.tile([128, 1], F32, tag="one")
            nc.gpsimd.memset(one[:], 1.0)
            halfpi = mp.tile([128, 1], F32, tag="halfpi")
            nc.gpsimd.memset(halfpi[:], float(np.pi / 2))

            # ---- unit vectors ----
            sq = mp.tile([128, 3, NCH], F32, tag="sq")
            nc.scalar.square(sq[:], v[:])
            n2 = mp.tile([128, NCH], F32, tag="n2")
            nc.vector.tensor_add(n2[:], sq[:, 0], sq[:, 1])
            nc.vector.tensor_add(n2[:], n2[:], sq[:, 2])
            nrm = mp.tile([128, NCH], F32, tag="nrm")
            nc.scalar.sqrt(nrm[:], n2[:])
            rn = mp.tile([128, NCH], F32, tag="rn")
            nc.vector.reciprocal(rn[:], nrm[:])
            u = mp.tile([128, 3, NCH], F32, tag="u")
            nc.vector.tensor_mul(u[:], v[:], rn[:].unsqueeze(1).broadcast_to([128, 3, NCH]))
            ux, uy, uz = u[:, 0], u[:, 1], u[:, 2]

            # ---- pair products (unscaled) ----
            t2 = mp.tile([128, 6, NCH], F32, tag="t2")
            pairs = [(0, 0), (0, 1), (0, 2), (1, 1), (1, 2), (2, 2)]
            for i, (a, b) in enumerate(pairs):
                nc.vector.tensor_mul(t2[:, i], u[:, a], u[:, b])
            txx, txy, txz, tyy, tyz, tzz = (t2[:, i] for i in range(6))

            # ---- angular basis (bf16, sqrt(pref) folded) ----
            ang = mp.tile([128, 20, NCH], BF16, tag="ang")
            nc.gpsimd.memset(ang[:, 0], 1.0)
            nc.scalar.copy(ang[:, 1], ux)
            nc.scalar.copy(ang[:, 2], uy)
            nc.scalar.copy(ang[:, 3], uz)
            l2 = [(txx, 1.0), (txy, S2), (txz, S2), (tyy, 1.0), (tyz, S2), (tzz, 1.0)]
            for i, (t, s) in enumerate(l2):
                nc.scalar.mul(ang[:, 4 + i], t, s)
            l3 = [(txx, 1.0, ux), (txx, S3, uy), (txx, S3, uz),
                  (tyy, S3, ux), (txy, S6, uz), (tzz, S3, ux),
                  (tyy, 1.0, uy), (tyy, S3, uz), (tzz, S3, uy), (tzz, 1.0, uz)]
            for i, (t, s, uu) in enumerate(l3):
                nc.vector.scalar_tensor_tensor(ang[:, 10 + i], t, s, uu,
                                               op0=ALU.mult, op1=ALU.mult)

            # ---- radial basis: sin(k*pi*r/C)/r * fc(r), k=1..8 ----
            R = mp.tile([128, 8, NCH], F32, tag="R")
            nc.scalar.activation(R[:, 0], r[:], AF.Sin, scale=float(np.pi / CUT))
            cs = mp.tile([128, NCH], F32, tag="cs")
            nc.scalar.activation(cs[:], r[:], AF.Sin, scale=float(-np.pi / CUT),
                                 bias=halfpi[:])
            nc.vector.scalar_tensor_tensor(R[:, 1], cs[:], 2.0, R[:, 0],
                                           op0=ALU.mult, op1=ALU.mult)
            for k in range(2, 8):
                nc.vector.scalar_tensor_tensor(R[:, k], cs[:], 2.0, R[:, k - 1],
                                               op0=ALU.mult, op1=ALU.mult)
                nc.vector.tensor_sub(R[:, k], R[:, k], R[:, k - 2])
            # cutoff polynomial fc = 1 - 28u^6 + 48u^7 - 21u^8, u = r/C
            uc = mp.tile([128, NCH], F32, tag="uc")
            nc.scalar.mul(uc[:], r[:], 1.0 / CUT)
            p2 = mp.tile([128, NCH], F32, tag="p2")
            nc.scalar.square(p2[:], uc[:])
            p3 = mp.tile([128, NCH], F32, tag="p3")
            nc.vector.tensor_mul(p3[:], p2[:], uc[:])
            p6 = mp.tile([128, NCH], F32, tag="p6")
            nc.scalar.square(p6[:], p3[:])
            p7 = mp.tile([128, NCH], F32, tag="p7")
            nc.vector.tensor_mul(p7[:], p6[:], uc[:])
            p8 = mp.tile([128, NCH], F32, tag="p8")
            nc.vector.tensor_mul(p8[:], p7[:], uc[:])
            fc = mp.tile([128, NCH], F32, tag="fc")
            nc.scalar.activation(fc[:], p6[:], AF.Identity, bias=one[:], scale=-28.0)
            nc.vector.scalar_tensor_tensor(fc[:], p7[:], 48.0, fc[:],
                                           op0=ALU.mult, op1=ALU.add)
            nc.vector.scalar_tensor_tensor(fc[:], p8[:], -21.0, fc[:],
                                           op0=ALU.mult, op1=ALU.add)
            msk = mp.tile([128, NCH], F32, tag="msk")
            nc.vector.tensor_scalar(msk[:], r[:], float(CUT), None, op0=ALU.is_lt)
            nc.vector.tensor_mul(fc[:], fc[:], msk[:])
            rinv = mp.tile([128, NCH], F32, tag="rinv")
            nc.vector.reciprocal(rinv[:], r[:])
            w = mp.tile([128, NCH], F32, tag="w")
            nc.vector.scalar_tensor_tensor(w[:], fc[:], SQ2C, rinv[:],
                                           op0=ALU.mult, op1=ALU.mult)
            nc.vector.tensor_mul(R[:], R[:], w[:].unsqueeze(1).broadcast_to([128, 8, NCH]))

            # ---- z-masked radial (bf16) ----
            m0 = mp.tile([128, NCH], F32, tag="m0")
            nc.scalar.activation(m0[:], z1[:], AF.Identity, bias=one[:], scale=-1.0)
            zR = mp.tile([128, 16, NCH], BF16, tag="zR")
            nc.vector.tensor_mul(zR[:, 0:8], R[:],
                                 m0[:].unsqueeze(1).broadcast_to([128, 8, NCH]))
            nc.vector.tensor_mul(zR[:, 8:16], R[:],
                                 z1[:].unsqueeze(1).broadcast_to([128, 8, NCH]))

            # ---- one-hot of receiver-within-block (bf16) ----
            oh = mp.tile([128, NCH, 128], BF16, tag="oh")
            nc.vector.tensor_tensor(
                oh[:],
                drel[:].unsqueeze(2).broadcast_to([128, NCH, 128]),
                iot[:].unsqueeze(1).broadcast_to([128, NCH, 128]),
                op=ALU.is_equal)

            # ---- emb^2 ----
            emb2 = mp.tile([128, NBLK, 3], F32, tag="emb2")
            nc.scalar.square(emb2[:], emb[:])

            # ---- segment-sum via one-hot matmul ----
            g_all = mp.tile([128, NBLK, 320], F32, tag="gall")
            for b in range(NBLK):
                g = ps.tile([128, 320], F32, tag="g")
                for k in range(CPB):
                    c = b * CPB + k
                    pt = pp.tile([128, 320], BF16, tag="P")
                    nc.vector.tensor_mul(
                        pt[:].rearrange("p (s a) -> p s a", s=16),
                        zR[:, :, c].unsqueeze(2).broadcast_to([128, 16, 20]),
                        ang[:, :, c].unsqueeze(1).broadcast_to([128, 16, 20]))
                    nc.tensor.matmul(g[:], oh[:, c], pt[:],
                                     start=(k == 0), stop=(k == CPB - 1))
                nc.scalar.copy(g_all[:, b], g[:])

            # ---- symmetrizer: M = sum_z G_z * W[z], then invariants ----
            gv = g_all[:].rearrange("p b (z ra) -> p b z ra", z=2)
            M = mp.tile([128, NBLK, 3, 160], F32, tag="M")
            Mt = mp.tile([128, NBLK, 3, 160], F32, tag="Mt")
            nc.vector.tensor_mul(
                M[:], gv[:, :, 0].unsqueeze(2).broadcast_to([128, NBLK, 3, 160]),
                wbc[:, 0:3].unsqueeze(1).unsqueeze(3).broadcast_to([128, NBLK, 3, 160]))
            nc.vector.tensor_mul(
                Mt[:], gv[:, :, 1].unsqueeze(2).broadcast_to([128, NBLK, 3, 160]),
                wbc[:, 3:6].unsqueeze(1).unsqueeze(3).broadcast_to([128, NBLK, 3, 160]))
            nc.vector.tensor_add(M[:], M[:], Mt[:])
            Ms = mp.tile([128, NBLK, 3, 160], F32, tag="Ms")
            nc.scalar.square(Ms[:], M[:])

            # fuse (block, c1) -> 30 so every op stays within 3 free dims
            M5 = M[:].rearrange("p b c (r a) -> p (b c) r a", r=8)
            Ms5 = Ms[:].rearrange("p b c (r a) -> p (b c) r a", r=8)
            SM = mp.tile([128, NBLK * 3, 8, 4], F32, tag="SM")
            nc.scalar.copy(SM[:, :, :, 0], M5[:, :, :, 0])
            nc.vector.tensor_reduce(SM[:, :, :, 1], Ms5[:, :, :, 1:4],
                                    axis=mybir.AxisListType.X, op=ALU.add)
            nc.vector.tensor_reduce(SM[:, :, :, 2], Ms5[:, :, :, 4:10],
                                    axis=mybir.AxisListType.X, op=ALU.add)
            nc.vector.tensor_reduce(SM[:, :, :, 3], Ms5[:, :, :, 10:20],
                                    axis=mybir.AxisListType.X, op=ALU.add)

            # ---- B[p, b, r, l, c1, c2] ----
            B = mp.tile([128, NBLK, 8, 4, 3, 3], F32, tag="B")
            SMv = SM[:].rearrange("p (b c) r s -> p b c r s", b=NBLK)
            for l in range(4):
                efac = emb if l == 0 else emb2
                for c1 in range(3):
                    nc.vector.tensor_mul(
                        B[:, :, :, l, c1],
                        SMv[:, :, c1, :, l].unsqueeze(3).broadcast_to(
                            [128, NBLK, 8, 3]),
                        efac[:].unsqueeze(2).broadcast_to([128, NBLK, 8, 3]))

            nc.sync.dma_start(
                o_d.ap(), B[:].rearrange("p b r l c d -> p (b r l c d)"))

    nc.compile()
    return nc


def _host_prep(inputs):
    an = np.asarray(inputs["atomic_numbers"]).astype(np.int64)
    ei = np.asarray(inputs["edge_index"]).astype(np.int64)
    el = np.asarray(inputs["edge_lengths"]).astype(np.float32)
    ev = np.asarray(inputs["edge_vectors"]).astype(np.float32)
    W = np.asarray(inputs["W_embed"]).astype(np.float32)
    bf16 = np.dtype("bfloat16") if "bfloat16" in np.sctypeDict else None
    import ml_dtypes
    bf16 = ml_dtypes.bfloat16

    emb = W[an]                                     # [N, 3]
    src, dst = ei[0], ei[1]
    z_src = an[src].astype(np.float32)
    order = np.argsort(dst, kind="stable")
    dst_s, el_s, ev_s, zs_s = dst[order], el[order], ev[order], z_src[order]

    iota = np.broadcast_to(np.arange(128, dtype=np.float32), (128, 128))
    iota16 = np.ascontiguousarray(iota.astype(bf16))
    wbc = np.broadcast_to(W.reshape(-1), (128, 6)).astype(np.float32)
    wbc = np.ascontiguousarray(wbc)

    S = NCH * 128
    in_maps = []
    for c in range(N_CORES):
        lo, hi = c * NPC, (c + 1) * NPC
        lo_i = np.searchsorted(dst_s, lo, "left")
        hi_i = np.searchsorted(dst_s, min(hi, N_NODES), "left")
        d_l = dst_s[lo_i:hi_i] - lo
        e_l, v_l, z_l = el_s[lo_i:hi_i], ev_s[lo_i:hi_i], zs_s[lo_i:hi_i]

        r_pad = np.ones(S, np.float32)
        v_pad = np.zeros((S, 3), np.float32)
        v_pad[:, 0] = 1.0
        z_pad = np.zeros(S, np.float32)
        drel_pad = np.full(S, -1.0, np.float32)
        blk = (d_l // 128).astype(np.int64)
        bounds = np.searchsorted(blk, np.arange(NBLK + 1), "left")
        for b in range(NBLK):
            s0, s1 = int(bounds[b]), int(bounds[b + 1])
            cnt = s1 - s0
            if cnt > CPB * 128:
                raise RuntimeError(f"core {c} block {b}: {cnt} edges > {CPB*128}")
            o = b * CPB * 128
            r_pad[o:o + cnt] = e_l[s0:s1]
            v_pad[o:o + cnt] = v_l[s0:s1]
            z_pad[o:o + cnt] = z_l[s0:s1]
            drel_pad[o:o + cnt] = (d_l[s0:s1] - b * 128).astype(np.float32)

        def lay(x):
            return np.ascontiguousarray(x.reshape(NCH, 128).T)

        v_lay = np.stack([lay(v_pad[:, 0]), lay(v_pad[:, 1]), lay(v_pad[:, 2])], 1)
        emb_core = np.zeros((NPC, 3), np.float32)
        n_real = max(0, min(hi, N_NODES) - lo)
        emb_core[:n_real] = emb[lo:lo + n_real]
        emb_lay = np.ascontiguousarray(
            emb_core.reshape(NBLK, 128, 3).transpose(1, 0, 2).reshape(128, NBLK * 3))

        in_maps.append(dict(
            r=lay(r_pad),
            v3=np.ascontiguousarray(v_lay.reshape(128, 3 * NCH)),
            z1=lay(z_pad),
            drel=np.ascontiguousarray(lay(drel_pad).astype(bf16)),
            iota=iota16,
            emb=emb_lay,
            wbc=wbc,
        ))
    return in_maps


def _make_runner(nc):
    """Cached-jit version of run_bass_kernel_spmd's axon execution path
    (bass2jax.run_bass_via_pjrt): one jitted shard_map over 8 NeuronCores,
    reused across kernel() calls instead of re-tracing every call."""
    import jax
    from concourse import bass2jax
    from jax.experimental.shard_map import shard_map
    from jax.sharding import Mesh, PartitionSpec

    bass2jax.install_neuronx_cc_hook()
    partition_name = (nc.partition_id_tensor.name
                      if nc.partition_id_tensor else None)
    in_names, out_names, out_avals = [], [], []
    for alloc in nc.m.functions[0].allocations:
        if not isinstance(alloc, mybir.MemoryLocationSet):
            continue
        name = alloc.memorylocations[0].name
        if alloc.kind == "ExternalInput":
            if name != partition_name:
                in_names.append(name)
        elif alloc.kind == "ExternalOutput":
            out_names.append(name)
            out_avals.append(jax.core.ShapedArray(
                tuple(alloc.tensor_shape), mybir.dt.np(alloc.dtype)))
    n_params, n_outs = len(in_names), len(out_names)
    all_in_names = list(in_names) + list(out_names)
    if partition_name is not None:
        all_in_names.append(partition_name)

    def _body(*args):
        operands = list(args)
        if partition_name is not None:
            operands.append(bass2jax.partition_id_tensor())
        outs = bass2jax._bass_exec_p.bind(
            *operands,
            out_avals=tuple(out_avals),
            in_names=tuple(all_in_names),
            out_names=tuple(out_names),
            lowering_input_output_aliases=(),
            sim_require_finite=True,
            sim_require_nnan=True,
            nc=nc)
        return tuple(outs)

    devices = jax.devices()[:N_CORES]
    mesh = Mesh(np.asarray(devices), ("core",))
    in_specs = (PartitionSpec("core"),) * (n_params + n_outs)
    out_specs = (PartitionSpec("core"),) * n_outs
    sharded = jax.jit(
        shard_map(_body, mesh=mesh, in_specs=in_specs, out_specs=out_specs,
                  check_rep=False),
        keep_unused=True)
    # zero output-seed buffers, resident on device, reused every call
    # (no donation, so they are never consumed)
    from jax.sharding import NamedSharding
    zero_outs = [
        jax.device_put(
            np.zeros((N_CORES * a.shape[0], *a.shape[1:]), a.dtype),
            NamedSharding(mesh, PartitionSpec("core")))
        for a in out_avals]
    return sharded, in_names, out_names, out_avals, zero_outs


def _run(in_maps):
    if "runner" not in _CACHE:
        _CACHE["nc"] = _build()
        _CACHE["runner"] = _make_runner(_CACHE["nc"])
    sharded, in_names, out_names, out_avals, zero_outs = _CACHE["runner"]
    concat_in = [np.concatenate([m[nm] for m in in_maps], 0) for nm in in_names]
    outs = sharded(*concat_in, *zero_outs)
    return np.asarray(outs[0])          # [8*128, 2880]


def kernel(**inputs):
    in_maps = _host_prep(inputs)
    raw = _run(in_maps)
    parts = []
    for c in range(N_CORES):
        o = raw[c * 128:(c + 1) * 128]               # [128, 2880]
        parts.append(o.reshape(128, NBLK, 288).transpose(1, 0, 2).reshape(NPC, 288))
    full = np.concatenate(parts, 0)[:N_NODES]
    return np.ascontiguousarray(full.reshape(N_NODES, N_RBF, 4, 9)).astype(np.float32)


# revision 14
# speedup vs baseline: 1.5375x; 1.5375x over previous
"""Trainium2 Bass kernel: CACE-style GNN message passing (nn_Cace_7155415515517).

Strategy (node-parallel, no collectives needed):
  - Host: sort edges by receiver node, shard receivers across 8 cores
    (1280 nodes each), lay edges out in 128-edge chunks grouped into
    128-node blocks (12 chunks per block, padded with null edges).
    For each embedding channel c1, the host also builds a W-scaled
    one-hot matrix ohw_c1[e, n] = (dst_e == n) * W_embed[z_src_e, c1]
    (bf16) which serves directly as the segment-sum matmul weights.
  - Device per core: radial basis via a Chebyshev sin recurrence
    (ACT Sin is only valid on [-pi, pi]), angular monomials with
    sqrt(multinomial) prefactors folded in, payload P[e, (r, a)] (160
    wide, bf16); per node block, 3 PSUM accumulations
    M[n, c1, r, a] = sum_e ohw_c1[e, n] * P[e, r, a]; then the
    node-local symmetrizer using A[n,r,a,c1,c2] = emb[n,c2]*M[n,r,a,c1]:
    B_l = (sum_{a in l} pref * M^2) * emb^2, B_0 = M[..,a=0,..]*emb.
"""
import math
import numpy as np

import concourse.bacc as bacc
import concourse.bass as bass
import concourse.mybir as mybir
import concourse.tile as tile

AF = mybir.ActivationFunctionType
ALU = mybir.AluOpType
F32 = mybir.dt.float32
BF16 = mybir.dt.bfloat16

N_CORES = 8
N_NODES = 10000
N_RBF = 8
NPC = 1280            # nodes per core (8*1280 = 10240, tail padded)
NBLK = 10             # 128-node blocks per core
CPB = 12              # 128-edge chunks per block
NCH = NBLK * CPB      # 120 chunks -> 15360 edge slots per core
CUT = 5.5
SQ2C = math.sqrt(2.0 / CUT)
S2, S3, S6 = math.sqrt(2.0), math.sqrt(3.0), math.sqrt(6.0)

_CACHE = {}


def _build():
    nc = bacc.Bacc("TRN2", target_bir_lowering=False, debug=False,
                   num_devices=N_CORES)
    r_d = nc.dram_tensor("r", [128, NCH], F32, kind="ExternalInput")
    v_d = nc.dram_tensor("v3", [128, 3 * NCH], F32, kind="ExternalInput")
    ohw_d = [nc.dram_tensor(f"ohw{c1}", [128, NCH * 128], BF16,
                            kind="ExternalInput") for c1 in range(3)]
    e_d = nc.dram_tensor("emb", [128, 3 * NBLK], F32, kind="ExternalInput")
    o_d = nc.dram_tensor("out", [128, 288 * NBLK], F32, kind="ExternalOutput")

    with tile.TileContext(nc) as tc:
        with (
            tc.tile_pool(name="mp", bufs=1) as mp,
            tc.tile_pool(name="pp", bufs=3) as pp,
            tc.tile_pool(name="ps", bufs=2, space="PSUM") as ps,
        ):
            # ---- input loads ----
            r = mp.tile([128, NCH], F32, tag="r")
            nc.sync.dma_start(r[:], r_d.ap())
            v = mp.tile([128, 3, NCH], F32, tag="v")
            nc.sync.dma_start(v[:], v_d.ap().rearrange("p (a c) -> p a c", a=3))
            ohw = []
            for c1 in range(3):
                t = mp.tile([128, NCH, 128], BF16, tag=f"ohw{c1}")
                # split into 4 DMAs so multiple queues move it in parallel
                q = NCH // 4
                for s in range(4):
                    nc.sync.dma_start(
                        t[:, s * q:(s + 1) * q],
                        ohw_d[c1].ap().rearrange("p (c n) -> p c n", n=128)
                        [:, s * q:(s + 1) * q])
                ohw.append(t)
            emb = mp.tile([128, NBLK, 3], F32, tag="emb")
            nc.sync.dma_start(emb[:], e_d.ap().rearrange("p (b c) -> p b c", b=NBLK))

            one = mp.tile([128, 1], F32, tag="one")
            nc.gpsimd.memset(one[:], 1.0)
            halfpi = mp.tile([128, 1], F32, tag="halfpi")
            nc.gpsimd.memset(halfpi[:], float(np.pi / 2))

            # ---- unit vectors ----
            sq = mp.tile([128, 3, NCH], F32, tag="sq")
            nc.scalar.square(sq[:], v[:])
            n2 = mp.tile([128, NCH], F32, tag="n2")
            nc.vector.tensor_add(n2[:], sq[:, 0], sq[:, 1])
            nc.vector.tensor_add(n2[:], n2[:], sq[:, 2])
            nrm = mp.tile([128, NCH], F32, tag="nrm")
            nc.scalar.sqrt(nrm[:], n2[:])
            rn = mp.tile([128, NCH], F32, tag="rn")
            nc.vector.reciprocal(rn[:], nrm[:])
            u = mp.tile([128, 3, NCH], F32, tag="u")
            nc.vector.tensor_mul(u[:], v[:], rn[:].unsqueeze(1).broadcast_to([128, 3, NCH]))
            ux, uy, uz = u[:, 0], u[:, 1], u[:, 2]

            # ---- pair products (unscaled, f32) ----
            t2 = mp.tile([128, 6, NCH], F32, tag="t2")
            pairs = [(0, 0), (0, 1), (0, 2), (1, 1), (1, 2), (2, 2)]
            for i, (a, b) in enumerate(pairs):
                nc.vector.tensor_mul(t2[:, i], u[:, a], u[:, b])
            txx, txy, txz, tyy, tyz, tzz = (t2[:, i] for i in range(6))

            # ---- angular basis (bf16, sqrt(pref) folded) ----
            ang = mp.tile([128, 20, NCH], BF16, tag="ang")
            nc.gpsimd.memset(ang[:, 0], 1.0)
            nc.scalar.copy(ang[:, 1], ux)
            nc.scalar.copy(ang[:, 2], uy)
            nc.scalar.copy(ang[:, 3], uz)
            l2 = [(txx, 1.0), (txy, S2), (txz, S2), (tyy, 1.0), (tyz, S2), (tzz, 1.0)]
            for i, (t, s) in enumerate(l2):
                nc.scalar.mul(ang[:, 4 + i], t, s)
            l3 = [(txx, 1.0, ux), (txx, S3, uy), (txx, S3, uz),
                  (tyy, S3, ux), (txy, S6, uz), (tzz, S3, ux),
                  (tyy, 1.0, uy), (tyy, S3, uz), (tzz, S3, uy), (tzz, 1.0, uz)]
            for i, (t, s, uu) in enumerate(l3):
                nc.vector.scalar_tensor_tensor(ang[:, 10 + i], t, s, uu,
                                               op0=ALU.mult, op1=ALU.mult)

            # ---- radial basis: sin(k*pi*r/C)/r * fc(r), k=1..8 ----
            R = mp.tile([128, 8, NCH], F32, tag="R")
            nc.scalar.activation(R[:, 0], r[:], AF.Sin, scale=float(np.pi / CUT))
            cs = mp.tile([128, NCH], F32, tag="cs")
            nc.scalar.activation(cs[:], r[:], AF.Sin, scale=float(-np.pi / CUT),
                                 bias=halfpi[:])
            nc.vector.scalar_tensor_tensor(R[:, 1], cs[:], 2.0, R[:, 0],
                                           op0=ALU.mult, op1=ALU.mult)
            for k in range(2, 8):
                nc.vector.scalar_tensor_tensor(R[:, k], cs[:], 2.0, R[:, k - 1],
                                               op0=ALU.mult, op1=ALU.mult)
                nc.vector.tensor_sub(R[:, k], R[:, k], R[:, k - 2])
            # cutoff polynomial fc = 1 - 28u^6 + 48u^7 - 21u^8, u = r/C
            uc = mp.tile([128, NCH], F32, tag="uc")
            nc.scalar.mul(uc[:], r[:], 1.0 / CUT)
            p2 = mp.tile([128, NCH], F32, tag="p2")
            nc.scalar.square(p2[:], uc[:])
            p3 = mp.tile([128, NCH], F32, tag="p3")
            nc.vector.tensor_mul(p3[:], p2[:], uc[:])
            p6 = mp.tile([128, NCH], F32, tag="p6")
            nc.scalar.square(p6[:], p3[:])
            p7 = mp.tile([128, NCH], F32, tag="p7")
            nc.vector.tensor_mul(p7[:], p6[:], uc[:])
            p8 = mp.tile([128, NCH], F32, tag="p8")
            nc.vector.tensor_mul(p8[:], p7[:], uc[:])
            fc = mp.tile([128, NCH], F32, tag="fc")
            nc.scalar.activation(fc[:], p6[:], AF.Identity, bias=one[:], scale=-28.0)
            nc.vector.scalar_tensor_tensor(fc[:], p7[:], 48.0, fc[:],
                                           op0=ALU.mult, op1=ALU.add)
            nc.vector.scalar_tensor_tensor(fc[:], p8[:], -21.0, fc[:],
                                           op0=ALU.mult, op1=ALU.add)
            msk = mp.tile([128, NCH], F32, tag="msk")
            nc.vector.tensor_scalar(msk[:], r[:], float(CUT), None, op0=ALU.is_lt)
            nc.vector.tensor_mul(fc[:], fc[:], msk[:])
            rinv = mp.tile([128, NCH], F32, tag="rinv")
            nc.vector.reciprocal(rinv[:], r[:])
            w = mp.tile([128, NCH], F32, tag="w")
            nc.vector.scalar_tensor_tensor(w[:], fc[:], SQ2C, rinv[:],
                                           op0=ALU.mult, op1=ALU.mult)
            nc.vector.tensor_mul(R[:], R[:], w[:].unsqueeze(1).broadcast_to([128, 8, NCH]))

            # ---- emb^2 ----
            emb2 = mp.tile([128, NBLK, 3], F32, tag="emb2")
            nc.scalar.square(emb2[:], emb[:])

            # ---- segment-sum via W-scaled one-hot matmuls ----
            # payload P[e, r, a] (160 wide) built one block per op; two
            # blocks on GpSimd to offload the DVE (bottleneck engine).
            m_all = mp.tile([128, NBLK, 3, 160], F32, tag="mall")
            for b in range(NBLK):
                c0 = b * CPB
                pt = pp.tile([128, CPB, 8, 20], BF16, tag="P")
                peng = nc.gpsimd if b >= NBLK - 4 else nc.vector
                peng.tensor_mul(
                    pt[:],
                    R[:, :, c0:c0 + CPB].transpose([0, 2, 1]).unsqueeze(3)
                        .broadcast_to([128, CPB, 8, 20]),
                    ang[:, :, c0:c0 + CPB].transpose([0, 2, 1]).unsqueeze(2)
                        .broadcast_to([128, CPB, 8, 20]))
                gs = [ps.tile([128, 160], F32, tag=f"g{c1}", name=f"g{c1}")
                      for c1 in range(3)]
                for k in range(CPB):
                    rhs = pt[:, k].rearrange("p s a -> p (s a)")
                    for c1 in range(3):
                        nc.tensor.matmul(
                            gs[c1][:], ohw[c1][:, c0 + k], rhs,
                            start=(k == 0), stop=(k == CPB - 1))
                for c1 in range(3):
                    nc.scalar.copy(m_all[:, b, c1], gs[c1][:])

            # ---- symmetrizer ----
            # fuse (block, c1) -> 30 so every op stays within 3 free dims
            M5 = m_all[:].rearrange("p b c (r a) -> p (b c) r a", r=8)
            SM = mp.tile([128, NBLK * 3, 8, 4], F32, tag="SM")
            nc.scalar.copy(SM[:, :, :, 0], M5[:, :, :, 0])
            Ms = mp.tile([128, NBLK, 3, 160], F32, tag="Ms")
            nc.scalar.square(Ms[:], m_all[:])
            Ms5 = Ms[:].rearrange("p b c (r a) -> p (b c) r a", r=8)
            nc.vector.tensor_reduce(SM[:, :, :, 1], Ms5[:, :, :, 1:4],
                                    axis=mybir.AxisListType.X, op=ALU.add)
            nc.vector.tensor_reduce(SM[:, :, :, 2], Ms5[:, :, :, 4:10],
                                    axis=mybir.AxisListType.X, op=ALU.add)
            nc.vector.tensor_reduce(SM[:, :, :, 3], Ms5[:, :, :, 10:20],
                                    axis=mybir.AxisListType.X, op=ALU.add)

            # ---- B[p, b, r, l, c1, c2] ----
            B = mp.tile([128, NBLK, 8, 4, 3, 3], F32, tag="B")
            SMv = SM[:].rearrange("p (b c) r s -> p b c r s", b=NBLK)
            for l in range(4):
                efac = emb if l == 0 else emb2
                for c1 in range(3):
                    nc.vector.tensor_mul(
                        B[:, :, :, l, c1],
                        SMv[:, :, c1, :, l].unsqueeze(3).broadcast_to(
                            [128, NBLK, 8, 3]),
                        efac[:].unsqueeze(2).broadcast_to([128, NBLK, 8, 3]))

            nc.sync.dma_start(
                o_d.ap(), B[:].rearrange("p b r l c d -> p (b r l c d)"))

    nc.compile()
    return nc


def _host_prep(inputs):
    import ml_dtypes
    bf16 = ml_dtypes.bfloat16

    an = np.asarray(inputs["atomic_numbers"]).astype(np.int64)
    ei = np.asarray(inputs["edge_index"]).astype(np.int64)
    el = np.asarray(inputs["edge_lengths"]).astype(np.float32)
    ev = np.asarray(inputs["edge_vectors"]).astype(np.float32)
    W = np.asarray(inputs["W_embed"]).astype(np.float32)

    emb = W[an]                                     # [N, 3]
    src, dst = ei[0], ei[1]
    z_src = an[src]
    order = np.argsort(dst, kind="stable")
    dst_s, el_s, ev_s, zs_s = dst[order], el[order], ev[order], z_src[order]
    Wz = W.astype(bf16)                             # [2, 3] in bf16

    in_maps = []
    for c in range(N_CORES):
        lo, hi = c * NPC, (c + 1) * NPC
        lo_i = np.searchsorted(dst_s, lo, "left")
        hi_i = np.searchsorted(dst_s, min(hi, N_NODES), "left")
        d_l = dst_s[lo_i:hi_i] - lo
        e_l, v_l, z_l = el_s[lo_i:hi_i], ev_s[lo_i:hi_i], zs_s[lo_i:hi_i]

        S = NCH * 128
        r_pad = np.ones(S, np.float32)
        v_pad = np.zeros((S, 3), np.float32)
        v_pad[:, 0] = 1.0
        # slot index for each real edge (block-padded layout)
        blk = (d_l // 128).astype(np.int64)
        bounds = np.searchsorted(blk, np.arange(NBLK + 1), "left")
        slot = np.empty(len(d_l), np.int64)
        for b in range(NBLK):
            s0, s1 = int(bounds[b]), int(bounds[b + 1])
            cnt = s1 - s0
            if cnt > CPB * 128:
                raise RuntimeError(f"core {c} block {b}: {cnt} edges > {CPB*128}")
            slot[s0:s1] = b * CPB * 128 + np.arange(cnt)
        r_pad[slot] = e_l
        v_pad[slot] = v_l

        # device layout [128, NCH]: edge i of chunk k at [i, k]
        def lay(x):
            return np.ascontiguousarray(x.reshape(NCH, 128).T)

        v_lay = np.stack([lay(v_pad[:, 0]), lay(v_pad[:, 1]), lay(v_pad[:, 2])], 1)

        # W-scaled one-hots: ohw[c1][e, chunk, n] = W[z_e, c1] at n = dst rel
        e_idx = slot % 128
        c_idx = slot // 128
        n_idx = d_l % 128
        ohw_list = []
        for c1 in range(3):
            arr = np.zeros((128, NCH, 128), bf16)
            arr[e_idx, c_idx, n_idx] = Wz[z_l, c1]
            ohw_list.append(arr.reshape(128, NCH * 128))

        emb_core = np.zeros((NPC, 3), np.float32)
        n_real = max(0, min(hi, N_NODES) - lo)
        emb_core[:n_real] = emb[lo:lo + n_real]
        emb_lay = np.ascontiguousarray(
            emb_core.reshape(NBLK, 128, 3).transpose(1, 0, 2).reshape(128, NBLK * 3))

        in_maps.append(dict(
            r=lay(r_pad),
            v3=np.ascontiguousarray(v_lay.reshape(128, 3 * NCH)),
            ohw0=ohw_list[0], ohw1=ohw_list[1], ohw2=ohw_list[2],
            emb=emb_lay,
        ))
    return in_maps


def _make_runner(nc):
    """Cached-jit version of run_bass_kernel_spmd's axon execution path
    (bass2jax.run_bass_via_pjrt): one jitted shard_map over 8 NeuronCores,
    reused across kernel() calls instead of re-tracing every call."""
    import jax
    from concourse import bass2jax
    from jax.experimental.shard_map import shard_map
    from jax.sharding import Mesh, PartitionSpec

    bass2jax.install_neuronx_cc_hook()
    partition_name = (nc.partition_id_tensor.name
                      if nc.partition_id_tensor else None)
    in_names, out_names, out_avals = [], [], []
    for alloc in nc.m.functions[0].allocations:
        if not isinstance(alloc, mybir.MemoryLocationSet):
            continue
        name = alloc.memorylocations[0].name
        if alloc.kind == "ExternalInput":
            if name != partition_name:
                in_names.append(name)
        elif alloc.kind == "ExternalOutput":
            out_names.append(name)
            out_avals.append(jax.core.ShapedArray(
                tuple(alloc.tensor_shape), mybir.dt.np(alloc.dtype)))
    n_params, n_outs = len(in_names), len(out_names)
    all_in_names = list(in_names) + list(out_names)
    if partition_name is not None:
        all_in_names.append(partition_name)

    def _body(*args):
        operands = list(args)
        if partition_name is not None:
            operands.append(bass2jax.partition_id_tensor())
        outs = bass2jax._bass_exec_p.bind(
            *operands,
            out_avals=tuple(out_avals),
            in_names=tuple(all_in_names),
            out_names=tuple(out_names),
            lowering_input_output_aliases=(),
            sim_require_finite=True,
            sim_require_nnan=True,
            nc=nc)
        return tuple(outs)

    devices = jax.devices()[:N_CORES]
    mesh = Mesh(np.asarray(devices), ("core",))
    in_specs = (PartitionSpec("core"),) * (n_params + n_outs)
    out_specs = (PartitionSpec("core"),) * n_outs
    sharded = jax.jit(
        shard_map(_body, mesh=mesh, in_specs=in_specs, out_specs=out_specs,
                  check_rep=False),
        keep_unused=True)
    # zero output-seed buffers, resident on device, reused every call
    # (no donation, so they are never consumed)
    from jax.sharding import NamedSharding
    zero_outs = [
        jax.device_put(
            np.zeros((N_CORES * a.shape[0], *a.shape[1:]), a.dtype),
            NamedSharding(mesh, PartitionSpec("core")))
        for a in out_avals]
    return sharded, in_names, out_names, out_avals, zero_outs


def _run(in_maps):
    if "runner" not in _CACHE:
        _CACHE["nc"] = _build()
        _CACHE["runner"] = _make_runner(_CACHE["nc"])
    sharded, in_names, out_names, out_avals, zero_outs = _CACHE["runner"]
    concat_in = [np.concatenate([m[nm] for m in in_maps], 0) for nm in in_names]
    outs = sharded(*concat_in, *zero_outs)
    return np.asarray(outs[0])          # [8*128, 2880]


def kernel(**inputs):
    in_maps = _host_prep(inputs)
    raw = _run(in_maps)
    parts = []
    for c in range(N_CORES):
        o = raw[c * 128:(c + 1) * 128]               # [128, 2880]
        parts.append(o.reshape(128, NBLK, 288).transpose(1, 0, 2).reshape(NPC, 288))
    full = np.concatenate(parts, 0)[:N_NODES]
    return np.ascontiguousarray(full.reshape(N_NODES, N_RBF, 4, 9)).astype(np.float32)


# revision 15
# speedup vs baseline: 1.6963x; 1.1033x over previous
"""Trainium2 Bass kernel: CACE-style GNN message passing (nn_Cace_7155415515517).

Strategy (node-parallel, no collectives needed):
  - Host: sort edges by receiver node, shard receivers across 8 cores
    (1280 nodes each), lay edges out in 128-edge chunks grouped into
    128-node blocks (12 chunks per block, padded with null edges).
    For each embedding channel c1, the host also builds a W-scaled
    one-hot matrix ohw_c1[e, n] = (dst_e == n) * W_embed[z_src_e, c1]
    (bf16) which serves directly as the segment-sum matmul weights.
  - Device per core: radial basis via a Chebyshev sin recurrence
    (ACT Sin is only valid on [-pi, pi]), angular monomials with
    sqrt(multinomial) prefactors folded in, payload P[e, (r, a)] (160
    wide, bf16); per node block, 3 PSUM accumulations
    M[n, c1, r, a] = sum_e ohw_c1[e, n] * P[e, r, a]; then the
    node-local symmetrizer using A[n,r,a,c1,c2] = emb[n,c2]*M[n,r,a,c1]:
    B_l = (sum_{a in l} pref * M^2) * emb^2, B_0 = M[..,a=0,..]*emb.
"""
import math
import numpy as np

import concourse.bacc as bacc
import concourse.bass as bass
import concourse.mybir as mybir
import concourse.tile as tile

AF = mybir.ActivationFunctionType
ALU = mybir.AluOpType
F32 = mybir.dt.float32
BF16 = mybir.dt.bfloat16
FP8 = mybir.dt.float8e4

N_CORES = 8
N_NODES = 10000
N_RBF = 8
NPC = 1280            # nodes per core (8*1280 = 10240, tail padded)
NBLK = 10             # 128-node blocks per core
CPB = 12              # 128-edge chunks per block
NCH = NBLK * CPB      # 120 chunks -> 15360 edge slots per core
CUT = 5.5
SQ2C = math.sqrt(2.0 / CUT)
S2, S3, S6 = math.sqrt(2.0), math.sqrt(3.0), math.sqrt(6.0)

_CACHE = {}


def _build():
    nc = bacc.Bacc("TRN2", target_bir_lowering=False, debug=False,
                   num_devices=N_CORES)
    r_d = nc.dram_tensor("r", [128, NCH], F32, kind="ExternalInput")
    v_d = nc.dram_tensor("v3", [128, 3 * NCH], F32, kind="ExternalInput")
    ohz_d = [nc.dram_tensor(f"ohz{z}", [128, NCH * 128], FP8,
                            kind="ExternalInput") for z in range(2)]
    e_d = nc.dram_tensor("emb", [128, 3 * NBLK], F32, kind="ExternalInput")
    w_d = nc.dram_tensor("wbc", [128, 6], F32, kind="ExternalInput")
    o_d = nc.dram_tensor("out", [128, 288 * NBLK], F32, kind="ExternalOutput")

    with tile.TileContext(nc) as tc:
        with (
            tc.tile_pool(name="mp", bufs=1) as mp,
            tc.tile_pool(name="pp", bufs=3) as pp,
            tc.tile_pool(name="ps", bufs=2, space="PSUM") as ps,
        ):
            # ---- input loads ----
            r = mp.tile([128, NCH], F32, tag="r")
            nc.sync.dma_start(r[:], r_d.ap())
            v = mp.tile([128, 3, NCH], F32, tag="v")
            nc.sync.dma_start(v[:], v_d.ap().rearrange("p (a c) -> p a c", a=3))
            ohz = []
            for z in range(2):
                t = mp.tile([128, NCH, 128], FP8, tag=f"ohz{z}", name=f"ohz{z}")
                # split into 4 DMAs so multiple queues move it in parallel
                q = NCH // 4
                for sp in range(4):
                    nc.sync.dma_start(
                        t[:, sp * q:(sp + 1) * q],
                        ohz_d[z].ap().rearrange("p (c n) -> p c n", n=128)
                        [:, sp * q:(sp + 1) * q])
                ohz.append(t)
            wbc = mp.tile([128, 6], F32, tag="wbc")
            nc.sync.dma_start(wbc[:], w_d.ap())
            emb = mp.tile([128, NBLK, 3], F32, tag="emb")
            nc.sync.dma_start(emb[:], e_d.ap().rearrange("p (b c) -> p b c", b=NBLK))

            one = mp.tile([128, 1], F32, tag="one")
            nc.gpsimd.memset(one[:], 1.0)
            halfpi = mp.tile([128, 1], F32, tag="halfpi")
            nc.gpsimd.memset(halfpi[:], float(np.pi / 2))

            # ---- unit vectors ----
            sq = mp.tile([128, 3, NCH], F32, tag="sq")
            nc.scalar.square(sq[:], v[:])
            n2 = mp.tile([128, NCH], F32, tag="n2")
            nc.vector.tensor_add(n2[:], sq[:, 0], sq[:, 1])
            nc.vector.tensor_add(n2[:], n2[:], sq[:, 2])
            nrm = mp.tile([128, NCH], F32, tag="nrm")
            nc.scalar.sqrt(nrm[:], n2[:])
            rn = mp.tile([128, NCH], F32, tag="rn")
            nc.vector.reciprocal(rn[:], nrm[:])
            u = mp.tile([128, 3, NCH], F32, tag="u")
            nc.vector.tensor_mul(u[:], v[:], rn[:].unsqueeze(1).broadcast_to([128, 3, NCH]))
            ux, uy, uz = u[:, 0], u[:, 1], u[:, 2]

            # ---- pair products (unscaled, f32) ----
            t2 = mp.tile([128, 6, NCH], F32, tag="t2")
            pairs = [(0, 0), (0, 1), (0, 2), (1, 1), (1, 2), (2, 2)]
            for i, (a, b) in enumerate(pairs):
                nc.vector.tensor_mul(t2[:, i], u[:, a], u[:, b])
            txx, txy, txz, tyy, tyz, tzz = (t2[:, i] for i in range(6))

            # ---- angular basis (bf16, sqrt(pref) folded) ----
            ang = mp.tile([128, 20, NCH], BF16, tag="ang")
            nc.gpsimd.memset(ang[:, 0], 1.0)
            nc.scalar.copy(ang[:, 1], ux)
            nc.scalar.copy(ang[:, 2], uy)
            nc.scalar.copy(ang[:, 3], uz)
            l2 = [(txx, 1.0), (txy, S2), (txz, S2), (tyy, 1.0), (tyz, S2), (tzz, 1.0)]
            for i, (t, s) in enumerate(l2):
                nc.scalar.mul(ang[:, 4 + i], t, s)
            l3 = [(txx, 1.0, ux), (txx, S3, uy), (txx, S3, uz),
                  (tyy, S3, ux), (txy, S6, uz), (tzz, S3, ux),
                  (tyy, 1.0, uy), (tyy, S3, uz), (tzz, S3, uy), (tzz, 1.0, uz)]
            for i, (t, s, uu) in enumerate(l3):
                nc.vector.scalar_tensor_tensor(ang[:, 10 + i], t, s, uu,
                                               op0=ALU.mult, op1=ALU.mult)

            # ---- radial basis: sin(k*pi*r/C)/r * fc(r), k=1..8 ----
            R = mp.tile([128, 8, NCH], F32, tag="R")
            nc.scalar.activation(R[:, 0], r[:], AF.Sin, scale=float(np.pi / CUT))
            cs = mp.tile([128, NCH], F32, tag="cs")
            nc.scalar.activation(cs[:], r[:], AF.Sin, scale=float(-np.pi / CUT),
                                 bias=halfpi[:])
            nc.vector.scalar_tensor_tensor(R[:, 1], cs[:], 2.0, R[:, 0],
                                           op0=ALU.mult, op1=ALU.mult)
            for k in range(2, 8):
                nc.vector.scalar_tensor_tensor(R[:, k], cs[:], 2.0, R[:, k - 1],
                                               op0=ALU.mult, op1=ALU.mult)
                nc.vector.tensor_sub(R[:, k], R[:, k], R[:, k - 2])
            # cutoff polynomial fc = 1 - 28u^6 + 48u^7 - 21u^8, u = r/C
            uc = mp.tile([128, NCH], F32, tag="uc")
            nc.scalar.mul(uc[:], r[:], 1.0 / CUT)
            p2 = mp.tile([128, NCH], F32, tag="p2")
            nc.scalar.square(p2[:], uc[:])
            p3 = mp.tile([128, NCH], F32, tag="p3")
            nc.vector.tensor_mul(p3[:], p2[:], uc[:])
            p6 = mp.tile([128, NCH], F32, tag="p6")
            nc.scalar.square(p6[:], p3[:])
            p7 = mp.tile([128, NCH], F32, tag="p7")
            nc.vector.tensor_mul(p7[:], p6[:], uc[:])
            p8 = mp.tile([128, NCH], F32, tag="p8")
            nc.vector.tensor_mul(p8[:], p7[:], uc[:])
            fc = mp.tile([128, NCH], F32, tag="fc")
            nc.scalar.activation(fc[:], p6[:], AF.Identity, bias=one[:], scale=-28.0)
            nc.vector.scalar_tensor_tensor(fc[:], p7[:], 48.0, fc[:],
                                           op0=ALU.mult, op1=ALU.add)
            nc.vector.scalar_tensor_tensor(fc[:], p8[:], -21.0, fc[:],
                                           op0=ALU.mult, op1=ALU.add)
            msk = mp.tile([128, NCH], F32, tag="msk")
            nc.vector.tensor_scalar(msk[:], r[:], float(CUT), None, op0=ALU.is_lt)
            nc.vector.tensor_mul(fc[:], fc[:], msk[:])
            rinv = mp.tile([128, NCH], F32, tag="rinv")
            nc.vector.reciprocal(rinv[:], r[:])
            w = mp.tile([128, NCH], F32, tag="w")
            nc.vector.scalar_tensor_tensor(w[:], fc[:], SQ2C, rinv[:],
                                           op0=ALU.mult, op1=ALU.mult)
            nc.vector.tensor_mul(R[:], R[:], w[:].unsqueeze(1).broadcast_to([128, 8, NCH]))

            # ---- emb^2 ----
            emb2 = mp.tile([128, NBLK, 3], F32, tag="emb2")
            nc.scalar.square(emb2[:], emb[:])

            # ---- segment-sum via z-masked one-hot matmuls ----
            # payload P[e, r, a] (160 wide) built one block per op; three
            # blocks on GpSimd to offload the DVE (bottleneck engine).
            g_all = mp.tile([128, NBLK, 2, 160], F32, tag="gall")
            for b in range(NBLK):
                c0 = b * CPB
                pt = pp.tile([128, CPB, 8, 20], BF16, tag="P")
                peng = nc.gpsimd if b >= NBLK - 3 else nc.vector
                peng.tensor_mul(
                    pt[:],
                    R[:, :, c0:c0 + CPB].transpose([0, 2, 1]).unsqueeze(3)
                        .broadcast_to([128, CPB, 8, 20]),
                    ang[:, :, c0:c0 + CPB].transpose([0, 2, 1]).unsqueeze(2)
                        .broadcast_to([128, CPB, 8, 20]))
                gs = [ps.tile([128, 160], F32, tag=f"g{z}", name=f"g{z}")
                      for z in range(2)]
                for k in range(CPB):
                    rhs = pt[:, k].rearrange("p s a -> p (s a)")
                    for z in range(2):
                        nc.tensor.matmul(
                            gs[z][:], ohz[z][:, c0 + k], rhs,
                            start=(k == 0), stop=(k == CPB - 1))
                for z in range(2):
                    nc.scalar.copy(g_all[:, b, z], gs[z][:])

            # ---- M[n, c1, r, a] = sum_z G_z * W[z, c1] ----
            # z=0 term on ACT (per-partition scale), z=1 accumulate on DVE
            m_all = mp.tile([128, NBLK, 3, 160], F32, tag="mall")
            for c1 in range(3):
                nc.scalar.mul(m_all[:, :, c1], g_all[:, :, 0],
                              wbc[:, c1:c1 + 1])
                nc.vector.scalar_tensor_tensor(
                    m_all[:, :, c1], g_all[:, :, 1], wbc[:, 3 + c1:4 + c1],
                    m_all[:, :, c1], op0=ALU.mult, op1=ALU.add)

            # ---- symmetrizer ----
            # fuse (block, c1) -> 30 so every op stays within 3 free dims
            M5 = m_all[:].rearrange("p b c (r a) -> p (b c) r a", r=8)
            SM = mp.tile([128, NBLK * 3, 8, 4], F32, tag="SM")
            nc.scalar.copy(SM[:, :, :, 0], M5[:, :, :, 0])
            Ms = mp.tile([128, NBLK, 3, 160], F32, tag="Ms")
            nc.scalar.square(Ms[:], m_all[:])
            Ms5 = Ms[:].rearrange("p b c (r a) -> p (b c) r a", r=8)
            nc.vector.tensor_reduce(SM[:, :, :, 1], Ms5[:, :, :, 1:4],
                                    axis=mybir.AxisListType.X, op=ALU.add)
            nc.vector.tensor_reduce(SM[:, :, :, 2], Ms5[:, :, :, 4:10],
                                    axis=mybir.AxisListType.X, op=ALU.add)
            nc.vector.tensor_reduce(SM[:, :, :, 3], Ms5[:, :, :, 10:20],
                                    axis=mybir.AxisListType.X, op=ALU.add)

            # ---- B[p, b, r, l, c1, c2] ----
            B = mp.tile([128, NBLK, 8, 4, 3, 3], F32, tag="B")
            SMv = SM[:].rearrange("p (b c) r s -> p b c r s", b=NBLK)
            for l in range(4):
                efac = emb if l == 0 else emb2
                for c1 in range(3):
                    nc.gpsimd.tensor_mul(
                        B[:, :, :, l, c1],
                        SMv[:, :, c1, :, l].unsqueeze(3).broadcast_to(
                            [128, NBLK, 8, 3]),
                        efac[:].unsqueeze(2).broadcast_to([128, NBLK, 8, 3]))

            nc.sync.dma_start(
                o_d.ap(), B[:].rearrange("p b r l c d -> p (b r l c d)"))

    nc.compile()
    return nc


def _host_prep(inputs):
    import ml_dtypes
    bf16 = ml_dtypes.bfloat16
    fp8 = ml_dtypes.float8_e4m3

    an = np.asarray(inputs["atomic_numbers"]).astype(np.int64)
    ei = np.asarray(inputs["edge_index"]).astype(np.int64)
    el = np.asarray(inputs["edge_lengths"]).astype(np.float32)
    ev = np.asarray(inputs["edge_vectors"]).astype(np.float32)
    W = np.asarray(inputs["W_embed"]).astype(np.float32)

    emb = W[an]                                     # [N, 3]
    src, dst = ei[0], ei[1]
    z_src = an[src]
    order = np.argsort(dst, kind="stable")
    dst_s, el_s, ev_s, zs_s = dst[order], el[order], ev[order], z_src[order]
    wbc = np.ascontiguousarray(
        np.broadcast_to(W.reshape(-1), (128, 6))).astype(np.float32)

    in_maps = []
    for c in range(N_CORES):
        lo, hi = c * NPC, (c + 1) * NPC
        lo_i = np.searchsorted(dst_s, lo, "left")
        hi_i = np.searchsorted(dst_s, min(hi, N_NODES), "left")
        d_l = dst_s[lo_i:hi_i] - lo
        e_l, v_l, z_l = el_s[lo_i:hi_i], ev_s[lo_i:hi_i], zs_s[lo_i:hi_i]

        S = NCH * 128
        r_pad = np.ones(S, np.float32)
        v_pad = np.zeros((S, 3), np.float32)
        v_pad[:, 0] = 1.0
        # slot index for each real edge (block-padded layout)
        blk = (d_l // 128).astype(np.int64)
        bounds = np.searchsorted(blk, np.arange(NBLK + 1), "left")
        slot = np.empty(len(d_l), np.int64)
        for b in range(NBLK):
            s0, s1 = int(bounds[b]), int(bounds[b + 1])
            cnt = s1 - s0
            if cnt > CPB * 128:
                raise RuntimeError(f"core {c} block {b}: {cnt} edges > {CPB*128}")
            slot[s0:s1] = b * CPB * 128 + np.arange(cnt)
        r_pad[slot] = e_l
        v_pad[slot] = v_l

        # device layout [128, NCH]: edge i of chunk k at [i, k]
        def lay(x):
            return np.ascontiguousarray(x.reshape(NCH, 128).T)

        v_lay = np.stack([lay(v_pad[:, 0]), lay(v_pad[:, 1]), lay(v_pad[:, 2])], 1)

        # z-masked one-hots: ohz[z][e, chunk, n] = (z_e == z) at n = dst rel
        e_idx = slot % 128
        c_idx = slot // 128
        n_idx = d_l % 128
        ohz_list = []
        for z in range(2):
            arr = np.zeros((128, NCH, 128), fp8)
            m = z_l == z
            arr[e_idx[m], c_idx[m], n_idx[m]] = 1.0
            ohz_list.append(arr.reshape(128, NCH * 128))

        emb_core = np.zeros((NPC, 3), np.float32)
        n_real = max(0, min(hi, N_NODES) - lo)
        emb_core[:n_real] = emb[lo:lo + n_real]
        emb_lay = np.ascontiguousarray(
            emb_core.reshape(NBLK, 128, 3).transpose(1, 0, 2).reshape(128, NBLK * 3))

        in_maps.append(dict(
            r=lay(r_pad),
            v3=np.ascontiguousarray(v_lay.reshape(128, 3 * NCH)),
            ohz0=ohz_list[0], ohz1=ohz_list[1],
            emb=emb_lay, wbc=wbc,
        ))
    return in_maps


def _make_runner(nc):
    """Cached-jit version of run_bass_kernel_spmd's axon execution path
    (bass2jax.run_bass_via_pjrt): one jitted shard_map over 8 NeuronCores,
    reused across kernel() calls instead of re-tracing every call."""
    import jax
    from concourse import bass2jax
    from jax.experimental.shard_map import shard_map
    from jax.sharding import Mesh, PartitionSpec

    bass2jax.install_neuronx_cc_hook()
    partition_name = (nc.partition_id_tensor.name
                      if nc.partition_id_tensor else None)
    in_names, out_names, out_avals = [], [], []
    for alloc in nc.m.functions[0].allocations:
        if not isinstance(alloc, mybir.MemoryLocationSet):
            continue
        name = alloc.memorylocations[0].name
        if alloc.kind == "ExternalInput":
            if name != partition_name:
                in_names.append(name)
        elif alloc.kind == "ExternalOutput":
            out_names.append(name)
            out_avals.append(jax.core.ShapedArray(
                tuple(alloc.tensor_shape), mybir.dt.np(alloc.dtype)))
    n_params, n_outs = len(in_names), len(out_names)
    all_in_names = list(in_names) + list(out_names)
    if partition_name is not None:
        all_in_names.append(partition_name)

    def _body(*args):
        operands = list(args)
        if partition_name is not None:
            operands.append(bass2jax.partition_id_tensor())
        outs = bass2jax._bass_exec_p.bind(
            *operands,
            out_avals=tuple(out_avals),
            in_names=tuple(all_in_names),
            out_names=tuple(out_names),
            lowering_input_output_aliases=(),
            sim_require_finite=True,
            sim_require_nnan=True,
            nc=nc)
        return tuple(outs)

    devices = jax.devices()[:N_CORES]
    mesh = Mesh(np.asarray(devices), ("core",))
    in_specs = (PartitionSpec("core"),) * (n_params + n_outs)
    out_specs = (PartitionSpec("core"),) * n_outs
    sharded = jax.jit(
        shard_map(_body, mesh=mesh, in_specs=in_specs, out_specs=out_specs,
                  check_rep=False),
        keep_unused=True)
    # zero output-seed buffers, resident on device, reused every call
    # (no donation, so they are never consumed)
    from jax.sharding import NamedSharding
    zero_outs = [
        jax.device_put(
            np.zeros((N_CORES * a.shape[0], *a.shape[1:]), a.dtype),
            NamedSharding(mesh, PartitionSpec("core")))
        for a in out_avals]
    return sharded, in_names, out_names, out_avals, zero_outs


def _run(in_maps):
    if "runner" not in _CACHE:
        _CACHE["nc"] = _build()
        _CACHE["runner"] = _make_runner(_CACHE["nc"])
    sharded, in_names, out_names, out_avals, zero_outs = _CACHE["runner"]
    concat_in = [np.concatenate([m[nm] for m in in_maps], 0) for nm in in_names]
    outs = sharded(*concat_in, *zero_outs)
    return np.asarray(outs[0])          # [8*128, 2880]


def kernel(**inputs):
    in_maps = _host_prep(inputs)
    raw = _run(in_maps)
    parts = []
    for c in range(N_CORES):
        o = raw[c * 128:(c + 1) * 128]               # [128, 2880]
        parts.append(o.reshape(128, NBLK, 288).transpose(1, 0, 2).reshape(NPC, 288))
    full = np.concatenate(parts, 0)[:N_NODES]
    return np.ascontiguousarray(full.reshape(N_NODES, N_RBF, 4, 9)).astype(np.float32)


# revision 17
# speedup vs baseline: 2.0308x; 1.1972x over previous
"""Trainium2 Bass kernel: CACE-style GNN message passing (nn_Cace_7155415515517).

Strategy (node-parallel, no collectives needed):
  - Host: sort edges by receiver node, shard receivers across 8 cores
    (1280 nodes each), lay edges out in 128-edge chunks grouped into
    128-node blocks (12 chunks per block, padded with null edges).
    For each embedding channel c1, the host also builds a W-scaled
    one-hot matrix ohw_c1[e, n] = (dst_e == n) * W_embed[z_src_e, c1]
    (bf16) which serves directly as the segment-sum matmul weights.
  - Device per core: radial basis via a Chebyshev sin recurrence
    (ACT Sin is only valid on [-pi, pi]), angular monomials with
    sqrt(multinomial) prefactors folded in, payload P[e, (r, a)] (160
    wide, bf16); per node block, 3 PSUM accumulations
    M[n, c1, r, a] = sum_e ohw_c1[e, n] * P[e, r, a]; then the
    node-local symmetrizer using A[n,r,a,c1,c2] = emb[n,c2]*M[n,r,a,c1]:
    B_l = (sum_{a in l} pref * M^2) * emb^2, B_0 = M[..,a=0,..]*emb.
"""
import math
import numpy as np

import concourse.bacc as bacc
import concourse.bass as bass
import concourse.mybir as mybir
import concourse.tile as tile

AF = mybir.ActivationFunctionType
ALU = mybir.AluOpType
F32 = mybir.dt.float32
BF16 = mybir.dt.bfloat16
FP8 = mybir.dt.float8e4

N_CORES = 8
N_NODES = 10000
N_RBF = 8
NPC = 1280            # nodes per core (8*1280 = 10240, tail padded)
NBLK = 10             # 128-node blocks per core
CPB = 12              # 128-edge chunks per block
NCH = NBLK * CPB      # 120 chunks -> 15360 edge slots per core
CUT = 5.5
SQ2C = math.sqrt(2.0 / CUT)
S2, S3, S6 = math.sqrt(2.0), math.sqrt(3.0), math.sqrt(6.0)

_CACHE = {}


def _build():
    nc = bacc.Bacc("TRN2", target_bir_lowering=False, debug=False,
                   num_devices=N_CORES)
    r_d = nc.dram_tensor("r", [128, NCH], F32, kind="ExternalInput")
    v_d = nc.dram_tensor("v3", [128, 3 * NCH], F32, kind="ExternalInput")
    ohz_d = [nc.dram_tensor(f"ohz{z}", [128, NCH * 128], FP8,
                            kind="ExternalInput") for z in range(2)]
    e_d = nc.dram_tensor("emb", [128, 3 * NBLK], F32, kind="ExternalInput")
    w_d = nc.dram_tensor("wbc", [128, 6], F32, kind="ExternalInput")
    o_d = nc.dram_tensor("out", [128, 288 * NBLK], F32, kind="ExternalOutput")

    with tile.TileContext(nc) as tc:
        with (
            tc.tile_pool(name="mp", bufs=1) as mp,
            tc.tile_pool(name="pp", bufs=3) as pp,
            tc.tile_pool(name="ps", bufs=2, space="PSUM") as ps,
        ):
            # ---- input loads ----
            r = mp.tile([128, NCH], F32, tag="r")
            nc.sync.dma_start(r[:], r_d.ap())
            v = mp.tile([128, 3, NCH], F32, tag="v")
            nc.sync.dma_start(v[:], v_d.ap().rearrange("p (a c) -> p a c", a=3))
            ohz = []
            for z in range(2):
                t = mp.tile([128, NCH, 128], FP8, tag=f"ohz{z}", name=f"ohz{z}")
                # split into 4 DMAs so multiple queues move it in parallel
                q = NCH // 4
                for sp in range(4):
                    nc.sync.dma_start(
                        t[:, sp * q:(sp + 1) * q],
                        ohz_d[z].ap().rearrange("p (c n) -> p c n", n=128)
                        [:, sp * q:(sp + 1) * q])
                ohz.append(t)
            wbc = mp.tile([128, 6], F32, tag="wbc")
            nc.sync.dma_start(wbc[:], w_d.ap())
            emb = mp.tile([128, NBLK, 3], F32, tag="emb")
            nc.sync.dma_start(emb[:], e_d.ap().rearrange("p (b c) -> p b c", b=NBLK))

            one = mp.tile([128, 1], F32, tag="one")
            nc.gpsimd.memset(one[:], 1.0)
            halfpi = mp.tile([128, 1], F32, tag="halfpi")
            nc.gpsimd.memset(halfpi[:], float(np.pi / 2))

            # ---- unit vectors ----
            sq = mp.tile([128, 3, NCH], F32, tag="sq")
            nc.scalar.square(sq[:], v[:])
            n2 = mp.tile([128, NCH], F32, tag="n2")
            nc.vector.tensor_add(n2[:], sq[:, 0], sq[:, 1])
            nc.vector.tensor_add(n2[:], n2[:], sq[:, 2])
            nrm = mp.tile([128, NCH], F32, tag="nrm")
            nc.scalar.sqrt(nrm[:], n2[:])
            rn = mp.tile([128, NCH], F32, tag="rn")
            nc.vector.reciprocal(rn[:], nrm[:])
            u = mp.tile([128, 3, NCH], F32, tag="u")
            nc.vector.tensor_mul(u[:], v[:], rn[:].unsqueeze(1).broadcast_to([128, 3, NCH]))
            ux, uy, uz = u[:, 0], u[:, 1], u[:, 2]

            # ---- pair products (unscaled, f32) ----
            t2 = mp.tile([128, 6, NCH], F32, tag="t2")
            pairs = [(0, 0), (0, 1), (0, 2), (1, 1), (1, 2), (2, 2)]
            for i, (a, b) in enumerate(pairs):
                nc.gpsimd.tensor_mul(t2[:, i], u[:, a], u[:, b])
            txx, txy, txz, tyy, tyz, tzz = (t2[:, i] for i in range(6))

            # ---- angular basis (bf16, sqrt(pref) folded) ----
            ang = mp.tile([128, 20, NCH], BF16, tag="ang")
            nc.gpsimd.memset(ang[:, 0], 1.0)
            nc.scalar.copy(ang[:, 1], ux)
            nc.scalar.copy(ang[:, 2], uy)
            nc.scalar.copy(ang[:, 3], uz)
            l2 = [(txx, 1.0), (txy, S2), (txz, S2), (tyy, 1.0), (tyz, S2), (tzz, 1.0)]
            for i, (t, s) in enumerate(l2):
                nc.scalar.mul(ang[:, 4 + i], t, s)
            l3 = [(txx, 1.0, ux), (txx, S3, uy), (txx, S3, uz),
                  (tyy, S3, ux), (txy, S6, uz), (tzz, S3, ux),
                  (tyy, 1.0, uy), (tyy, S3, uz), (tzz, S3, uy), (tzz, 1.0, uz)]
            for i, (t, s, uu) in enumerate(l3):
                nc.vector.scalar_tensor_tensor(ang[:, 10 + i], t, s, uu,
                                               op0=ALU.mult, op1=ALU.mult)

            # ---- radial basis: sin(k*pi*r/C)/r * fc(r), k=1..8 ----
            R = mp.tile([128, 8, NCH], F32, tag="R")
            nc.scalar.activation(R[:, 0], r[:], AF.Sin, scale=float(np.pi / CUT))
            cs = mp.tile([128, NCH], F32, tag="cs")
            nc.scalar.activation(cs[:], r[:], AF.Sin, scale=float(-np.pi / CUT),
                                 bias=halfpi[:])
            nc.vector.scalar_tensor_tensor(R[:, 1], cs[:], 2.0, R[:, 0],
                                           op0=ALU.mult, op1=ALU.mult)
            for k in range(2, 8):
                nc.vector.scalar_tensor_tensor(R[:, k], cs[:], 2.0, R[:, k - 1],
                                               op0=ALU.mult, op1=ALU.mult)
                nc.vector.tensor_sub(R[:, k], R[:, k], R[:, k - 2])
            # cutoff polynomial fc = 1 - 28u^6 + 48u^7 - 21u^8, u = r/C
            uc = mp.tile([128, NCH], F32, tag="uc")
            nc.scalar.mul(uc[:], r[:], 1.0 / CUT)
            p2 = mp.tile([128, NCH], F32, tag="p2")
            nc.scalar.square(p2[:], uc[:])
            p3 = mp.tile([128, NCH], F32, tag="p3")
            nc.gpsimd.tensor_mul(p3[:], p2[:], uc[:])
            p6 = mp.tile([128, NCH], F32, tag="p6")
            nc.scalar.square(p6[:], p3[:])
            p7 = mp.tile([128, NCH], F32, tag="p7")
            nc.gpsimd.tensor_mul(p7[:], p6[:], uc[:])
            p8 = mp.tile([128, NCH], F32, tag="p8")
            nc.gpsimd.tensor_mul(p8[:], p7[:], uc[:])
            fc = mp.tile([128, NCH], F32, tag="fc")
            nc.scalar.activation(fc[:], p6[:], AF.Identity, bias=one[:], scale=-28.0)
            nc.vector.scalar_tensor_tensor(fc[:], p7[:], 48.0, fc[:],
                                           op0=ALU.mult, op1=ALU.add)
            nc.vector.scalar_tensor_tensor(fc[:], p8[:], -21.0, fc[:],
                                           op0=ALU.mult, op1=ALU.add)
            msk = mp.tile([128, NCH], F32, tag="msk")
            nc.vector.tensor_scalar(msk[:], r[:], float(CUT), None, op0=ALU.is_lt)
            nc.vector.tensor_mul(fc[:], fc[:], msk[:])
            rinv = mp.tile([128, NCH], F32, tag="rinv")
            nc.vector.reciprocal(rinv[:], r[:])
            w = mp.tile([128, NCH], F32, tag="w")
            nc.vector.scalar_tensor_tensor(w[:], fc[:], SQ2C, rinv[:],
                                           op0=ALU.mult, op1=ALU.mult)
            nc.vector.tensor_mul(R[:], R[:], w[:].unsqueeze(1).broadcast_to([128, 8, NCH]))

            # ---- emb^2 ----
            emb2 = mp.tile([128, NBLK, 3], F32, tag="emb2")
            nc.scalar.square(emb2[:], emb[:])

            # ---- segment-sum via z-masked one-hot matmuls ----
            # payload P[e, r, a] (160 wide) built one block per op; three
            # blocks on GpSimd to offload the DVE (bottleneck engine).
            g_all = mp.tile([128, NBLK, 2, 160], F32, tag="gall")
            for b in range(NBLK):
                c0 = b * CPB
                pt = pp.tile([128, CPB, 8, 20], BF16, tag="P")
                peng = nc.gpsimd if b >= NBLK - 3 else nc.vector
                peng.tensor_mul(
                    pt[:],
                    R[:, :, c0:c0 + CPB].transpose([0, 2, 1]).unsqueeze(3)
                        .broadcast_to([128, CPB, 8, 20]),
                    ang[:, :, c0:c0 + CPB].transpose([0, 2, 1]).unsqueeze(2)
                        .broadcast_to([128, CPB, 8, 20]))
                gs = [ps.tile([128, 160], F32, tag=f"g{z}", name=f"g{z}")
                      for z in range(2)]
                for k in range(CPB):
                    rhs = pt[:, k].rearrange("p s a -> p (s a)")
                    for z in range(2):
                        nc.tensor.matmul(
                            gs[z][:], ohz[z][:, c0 + k], rhs,
                            start=(k == 0), stop=(k == CPB - 1))
                for z in range(2):
                    nc.scalar.copy(g_all[:, b, z], gs[z][:])

            # ---- post-stage: symmetrizer, done in two block-halves so
            # the first half overlaps the second half's matmuls ----
            m_all = mp.tile([128, NBLK, 3, 160], F32, tag="mall")
            Ms = mp.tile([128, NBLK, 3, 160], F32, tag="Ms")
            SM = mp.tile([128, NBLK * 3, 8, 4], F32, tag="SM")
            B = mp.tile([128, NBLK, 8, 4, 3, 3], F32, tag="B")
            M5 = m_all[:].rearrange("p b c (r a) -> p (b c) r a", r=8)
            Ms5 = Ms[:].rearrange("p b c (r a) -> p (b c) r a", r=8)
            SMv = SM[:].rearrange("p (b c) r s -> p b c r s", b=NBLK)
            HB = NBLK // 2
            for h in range(2):
                bs = slice(h * HB, (h + 1) * HB)
                fs = slice(h * HB * 3, (h + 1) * HB * 3)   # fused (b c1) rows
                # M[n, c1, r, a] = sum_z G_z * W[z, c1]
                for c1 in range(3):
                    nc.scalar.mul(m_all[:, bs, c1], g_all[:, bs, 0],
                                  wbc[:, c1:c1 + 1])
                    nc.vector.scalar_tensor_tensor(
                        m_all[:, bs, c1], g_all[:, bs, 1],
                        wbc[:, 3 + c1:4 + c1],
                        m_all[:, bs, c1], op0=ALU.mult, op1=ALU.add)
                nc.scalar.copy(SM[:, fs, :, 0], M5[:, fs, :, 0])
                nc.scalar.square(Ms[:, bs], m_all[:, bs])
                nc.vector.tensor_reduce(SM[:, fs, :, 1], Ms5[:, fs, :, 1:4],
                                        axis=mybir.AxisListType.X, op=ALU.add)
                nc.vector.tensor_reduce(SM[:, fs, :, 2], Ms5[:, fs, :, 4:10],
                                        axis=mybir.AxisListType.X, op=ALU.add)
                nc.vector.tensor_reduce(SM[:, fs, :, 3], Ms5[:, fs, :, 10:20],
                                        axis=mybir.AxisListType.X, op=ALU.add)
                # B[p, b, r, l, c1, c2]
                for l in range(4):
                    efac = emb if l == 0 else emb2
                    for c1 in range(3):
                        nc.gpsimd.tensor_mul(
                            B[:, bs, :, l, c1],
                            SMv[:, bs, c1, :, l].unsqueeze(3).broadcast_to(
                                [128, HB, 8, 3]),
                            efac[:, bs].unsqueeze(2).broadcast_to([128, HB, 8, 3]))
                nc.sync.dma_start(
                    o_d.ap()[:, h * HB * 288:(h + 1) * HB * 288],
                    B[:, bs].rearrange("p b r l c d -> p (b r l c d)"))

    nc.compile()
    return nc


def _host_prep(inputs):
    import ml_dtypes
    bf16 = ml_dtypes.bfloat16
    fp8 = ml_dtypes.float8_e4m3

    an = np.asarray(inputs["atomic_numbers"]).astype(np.int64)
    ei = np.asarray(inputs["edge_index"]).astype(np.int64)
    el = np.asarray(inputs["edge_lengths"]).astype(np.float32)
    ev = np.asarray(inputs["edge_vectors"]).astype(np.float32)
    W = np.asarray(inputs["W_embed"]).astype(np.float32)

    emb = W[an]                                     # [N, 3]
    src, dst = ei[0], ei[1]
    z_src = an[src]
    order = np.argsort(dst, kind="stable")
    dst_s, el_s, ev_s, zs_s = dst[order], el[order], ev[order], z_src[order]
    wbc = np.ascontiguousarray(
        np.broadcast_to(W.reshape(-1), (128, 6))).astype(np.float32)

    in_maps = []
    for c in range(N_CORES):
        lo, hi = c * NPC, (c + 1) * NPC
        lo_i = np.searchsorted(dst_s, lo, "left")
        hi_i = np.searchsorted(dst_s, min(hi, N_NODES), "left")
        d_l = dst_s[lo_i:hi_i] - lo
        e_l, v_l, z_l = el_s[lo_i:hi_i], ev_s[lo_i:hi_i], zs_s[lo_i:hi_i]

        S = NCH * 128
        r_pad = np.ones(S, np.float32)
        v_pad = np.zeros((S, 3), np.float32)
        v_pad[:, 0] = 1.0
        # slot index for each real edge (block-padded layout)
        blk = (d_l // 128).astype(np.int64)
        bounds = np.searchsorted(blk, np.arange(NBLK + 1), "left")
        slot = np.empty(len(d_l), np.int64)
        for b in range(NBLK):
            s0, s1 = int(bounds[b]), int(bounds[b + 1])
            cnt = s1 - s0
            if cnt > CPB * 128:
                raise RuntimeError(f"core {c} block {b}: {cnt} edges > {CPB*128}")
            slot[s0:s1] = b * CPB * 128 + np.arange(cnt)
        r_pad[slot] = e_l
        v_pad[slot] = v_l

        # device layout [128, NCH]: edge i of chunk k at [i, k]
        def lay(x):
            return np.ascontiguousarray(x.reshape(NCH, 128).T)

        v_lay = np.stack([lay(v_pad[:, 0]), lay(v_pad[:, 1]), lay(v_pad[:, 2])], 1)

        # z-masked one-hots: ohz[z][e, chunk, n] = (z_e == z) at n = dst rel
        e_idx = slot % 128
        c_idx = slot // 128
        n_idx = d_l % 128
        ohz_list = []
        for z in range(2):
            arr = np.zeros((128, NCH, 128), fp8)
            m = z_l == z
            arr[e_idx[m], c_idx[m], n_idx[m]] = 1.0
            ohz_list.append(arr.reshape(128, NCH * 128))

        emb_core = np.zeros((NPC, 3), np.float32)
        n_real = max(0, min(hi, N_NODES) - lo)
        emb_core[:n_real] = emb[lo:lo + n_real]
        emb_lay = np.ascontiguousarray(
            emb_core.reshape(NBLK, 128, 3).transpose(1, 0, 2).reshape(128, NBLK * 3))

        in_maps.append(dict(
            r=lay(r_pad),
            v3=np.ascontiguousarray(v_lay.reshape(128, 3 * NCH)),
            ohz0=ohz_list[0], ohz1=ohz_list[1],
            emb=emb_lay, wbc=wbc,
        ))
    return in_maps


def _make_runner(nc):
    """Cached-jit version of run_bass_kernel_spmd's axon execution path
    (bass2jax.run_bass_via_pjrt): one jitted shard_map over 8 NeuronCores,
    reused across kernel() calls instead of re-tracing every call."""
    import jax
    from concourse import bass2jax
    from jax.experimental.shard_map import shard_map
    from jax.sharding import Mesh, PartitionSpec

    bass2jax.install_neuronx_cc_hook()
    partition_name = (nc.partition_id_tensor.name
                      if nc.partition_id_tensor else None)
    in_names, out_names, out_avals = [], [], []
    for alloc in nc.m.functions[0].allocations:
        if not isinstance(alloc, mybir.MemoryLocationSet):
            continue
        name = alloc.memorylocations[0].name
        if alloc.kind == "ExternalInput":
            if name != partition_name:
                in_names.append(name)
        elif alloc.kind == "ExternalOutput":
            out_names.append(name)
            out_avals.append(jax.core.ShapedArray(
                tuple(alloc.tensor_shape), mybir.dt.np(alloc.dtype)))
    n_params, n_outs = len(in_names), len(out_names)
    all_in_names = list(in_names) + list(out_names)
    if partition_name is not None:
        all_in_names.append(partition_name)

    def _body(*args):
        operands = list(args)
        if partition_name is not None:
            operands.append(bass2jax.partition_id_tensor())
        outs = bass2jax._bass_exec_p.bind(
            *operands,
            out_avals=tuple(out_avals),
            in_names=tuple(all_in_names),
            out_names=tuple(out_names),
            lowering_input_output_aliases=(),
            sim_require_finite=True,
            sim_require_nnan=True,
            nc=nc)
        return tuple(outs)

    devices = jax.devices()[:N_CORES]
    mesh = Mesh(np.asarray(devices), ("core",))
    in_specs = (PartitionSpec("core"),) * (n_params + n_outs)
    out_specs = (PartitionSpec("core"),) * n_outs
    sharded = jax.jit(
        shard_map(_body, mesh=mesh, in_specs=in_specs, out_specs=out_specs,
                  check_rep=False),
        keep_unused=True)
    # zero output-seed buffers, resident on device, reused every call
    # (no donation, so they are never consumed)
    from jax.sharding import NamedSharding
    zero_outs = [
        jax.device_put(
            np.zeros((N_CORES * a.shape[0], *a.shape[1:]), a.dtype),
            NamedSharding(mesh, PartitionSpec("core")))
        for a in out_avals]
    return sharded, in_names, out_names, out_avals, zero_outs


def _run(in_maps):
    if "runner" not in _CACHE:
        _CACHE["nc"] = _build()
        _CACHE["runner"] = _make_runner(_CACHE["nc"])
    sharded, in_names, out_names, out_avals, zero_outs = _CACHE["runner"]
    concat_in = [np.concatenate([m[nm] for m in in_maps], 0) for nm in in_names]
    outs = sharded(*concat_in, *zero_outs)
    return np.asarray(outs[0])          # [8*128, 2880]


def kernel(**inputs):
    in_maps = _host_prep(inputs)
    raw = _run(in_maps)
    parts = []
    for c in range(N_CORES):
        o = raw[c * 128:(c + 1) * 128]               # [128, 2880]
        parts.append(o.reshape(128, NBLK, 288).transpose(1, 0, 2).reshape(NPC, 288))
    full = np.concatenate(parts, 0)[:N_NODES]
    return np.ascontiguousarray(full.reshape(N_NODES, N_RBF, 4, 9)).astype(np.float32)
